# revision 17
# baseline (speedup 1.0000x reference)
"""Dissipative Hamiltonian derivation — Trainium2 Bass kernel, 8-core SPMD.

Math (derived analytically from the jax reference; gradients computed in
closed form, no autodiff):
  vs = sigmoid(v); vq = [vs, q]; R = vq @ W1_w.T; U = R + b
  S[i,j] = ||r_i||^2 + ||u_j||^2 - 2 r_i.u_j          (= ||u_j - r_i||^2)
  dist = softplus(S); C = 2*mask*(dist^-2 - 2*dist^-3)*sigmoid(S)
  mask = (mvw*m).T @ (mvw*m)
  B[i] = (C @ U)[i] - rowsum(C)[i]*r_i        (local to the row shard)
  A[j] = colsum(C)[j]*u_j - (C.T @ R)[j]      (needs cross-core reduction)
  dHdq = (A - B) @ W1_w[:, 64:]
  (diagonal of C cancels exactly in A - B, so it is never zeroed)
  dq = dHdp = (2/m)*(softplus(zT)*sigmoid(zT)) @ W_T[:, 64:],  zT = [vs,p]@W_T.T
  dp = -(dHdq + (2/m)*(softplus(zF)*sigmoid(zF)) @ W_F),        zF = p@W_F.T

Sharding: rows of the N^2 pairwise computation, 192 rows per core.
Each core computes C for its 192 rows, reduces B locally, and contributes
P[j] = sum_{i in shard} c_ij*[r_i | 1] which is ReduceScatter-summed so
core c receives the slab of A-partials for its own 192 output rows.
"""

import os
import numpy as np

N = 1536
NCORES = 8
SH = N // NCORES            # 192 rows per core
H = 16
VD = 64
ITILES = [(0, 128), (128, 64)]   # i-tiles inside a shard (partition dim <= 128)
NJ = N // 128                # 12 j-chunks of 128
NJ3 = N // 512               # 3 j-chunks of 512

_CACHE = {}


def _build_nc():
    from concourse import bacc, mybir
    import concourse.tile as tile

    f32 = mybir.dt.float32
    AF = mybir.ActivationFunctionType

    # Bacc (not raw Bass): its compile() pipeline splits multi-sem waits
    # (move_matmul_waits_to_ldweights / generate_event_semaphores), which
    # TRN2 codegen requires for Tile-generated programs.
    nc = bacc.Bacc(None, num_devices=NCORES)

    def ein(name, shape):
        return nc.dram_tensor(name, shape, f32, kind="ExternalInput")

    vqT_d = ein("vqT", [96, N])       # [vs; q].T replicated
    vqTs_d = ein("vqTs", [96, SH])    # shard columns of vqT
    vpTs_d = ein("vpTs", [96, SH])    # [vs; p].T shard columns
    pTs_d = ein("pTs", [32, SH])
    m_d = ein("m_s", [SH, 1])
    mvwm_d = ein("mvwm", [48, N])     # mvw * m (mask factor), replicated
    mvwms_d = ein("mvwms", [48, SH])
    W1wT_d = ein("W1wT", [96, H])
    W1b_d = ein("W1b", [H, 1])
    W1q_d = ein("W1q", [H, 32])
    WTT_d = ein("WTT", [96, H])
    WTp_d = ein("WTp", [H, 32])
    WFT_d = ein("WFT", [32, H])
    WFm_d = ein("WFm", [H, 32])
    id_d = ein("ident", [128, 128])
    ones_d = ein("ones_row", [1, N])

    dp_d = nc.dram_tensor("dp_s", [SH, 32], f32, kind="ExternalOutput")
    dq_d = nc.dram_tensor("dq_s", [SH, 32], f32, kind="ExternalOutput")

    with tile.TileContext(nc) as tc:
        with (
            tc.tile_pool(name="const", bufs=1) as cp,
            tc.tile_pool(name="work", bufs=3) as wp,
            tc.tile_pool(name="dram", bufs=1, space="DRAM") as drp,
        ):
            def load(d, shape, tag):
                t = cp.tile(shape, f32, tag=tag)
                nc.sync.dma_start(t[:], d[:])
                return t

            vqT = load(vqT_d, [96, N], "vqT")
            vqTs = load(vqTs_d, [96, SH], "vqTs")
            vpTs = load(vpTs_d, [96, SH], "vpTs")
            pTs = load(pTs_d, [32, SH], "pTs")
            mvwm = load(mvwm_d, [48, N], "mvwm")
            mvwms = load(mvwms_d, [48, SH], "mvwms")
            W1wT = load(W1wT_d, [96, H], "W1wT")
            W1b = load(W1b_d, [H, 1], "W1b")
            W1q = load(W1q_d, [H, 32], "W1q")
            WTT = load(WTT_d, [96, H], "WTT")
            WTp = load(WTp_d, [H, 32], "WTp")
            WFT = load(WFT_d, [32, H], "WFT")
            WFm = load(WFm_d, [H, 32], "WFm")
            ident = load(id_d, [128, 128], "ident")

            UTx = cp.tile([H, N], f32, tag="UTx")      # U.T
            UTxX = cp.tile([2, N], f32, tag="UTxX")    # [ones; un2]
            ut2 = cp.tile([H, N], f32, tag="ut2")
            Slhs = cp.tile([H, SH], f32, tag="Slhs")   # -2 R.T
            SlhsX = cp.tile([2, SH], f32, tag="SlhsX")  # [rn2; ones]
            ones16 = cp.tile([H, 1], f32, tag="ones16")
            rts = cp.tile([H, SH], f32, tag="rts")     # R.T shard cols
            uts = cp.tile([H, SH], f32, tag="uts")     # U.T shard cols
            uro = cp.tile([128, 17 * NJ], f32, tag="uro")  # U rows | 1, per j-chunk
            rro0 = cp.tile([128, 17], f32, tag="rro0")     # R rows | 1, shard
            rro1 = cp.tile([64, 17], f32, tag="rro1")
            urs0 = cp.tile([128, H], f32, tag="urs0")      # U rows, shard
            urs1 = cp.tile([64, H], f32, tag="urs1")
            c0 = cp.tile([128, N], f32, tag="c0")
            c1 = cp.tile([64, N], f32, tag="c1")

            P_dram = drp.tile([N, 17], f32)
            P_red = drp.tile([SH, 17], f32)

            nc.vector.memset(ones16[:], 1.0)

            with tc.tile_pool(name="pss", bufs=4, space="PSUM") as pss:
                # U.T = R.T + b, full N
                for k in range(NJ3):
                    ps = pss.tile([H, 512], f32, tag="set")
                    nc.tensor.matmul(ps[:], W1wT[:], vqT[:, k * 512:(k + 1) * 512],
                                     start=True, stop=True)
                    nc.vector.tensor_scalar_add(UTx[:, k * 512:(k + 1) * 512],
                                                ps[:], W1b[:])
                # auxiliary rows [ones; un2] live in their own 2-partition tile
                # (DMA-written: partition 1 is off the quad boundary)
                nc.sync.dma_start(UTxX[0:1, :], ones_d[:, :])
                # un2 row = colwise ||u||^2
                nc.vector.tensor_mul(ut2[:], UTx[:], UTx[:])
                for k in range(NJ3):
                    ps = pss.tile([1, 512], f32, tag="set")
                    nc.tensor.matmul(ps[:], ones16[:], ut2[:, k * 512:(k + 1) * 512],
                                     start=True, stop=True)
                    tmp = wp.tile([1, 512], f32, tag="row")
                    nc.scalar.copy(tmp[:], ps[:])
                    nc.sync.dma_start(UTxX[1:2, k * 512:(k + 1) * 512], tmp[:])
                # R.T shard cols
                ps = pss.tile([H, SH], f32, tag="set")
                nc.tensor.matmul(ps[:], W1wT[:], vqTs[:], start=True, stop=True)
                nc.scalar.copy(rts[:], ps[:])
                nc.vector.tensor_scalar_add(uts[:], rts[:], W1b[:])
                # S lhsT main = -2 R.T ; aux rows = [rn2; ones]
                nc.scalar.mul(Slhs[:], rts[:], -2.0)
                rts2 = wp.tile([H, SH], f32, tag="rts2")
                nc.vector.tensor_mul(rts2[:], rts[:], rts[:])
                ps = pss.tile([1, SH], f32, tag="set")
                nc.tensor.matmul(ps[:], ones16[:], rts2[:], start=True, stop=True)
                tmp = wp.tile([1, SH], f32, tag="row2")
                nc.scalar.copy(tmp[:], ps[:])
                nc.sync.dma_start(SlhsX[0:1, :], tmp[:])
                nc.sync.dma_start(SlhsX[1:2, :], ones_d[:, 0:SH])
                # U rows (all N, by 128-chunk) and R/U rows for the shard
                for jc in range(NJ):
                    ps = pss.tile([128, H], f32, tag="tr")
                    nc.tensor.transpose(ps[:], UTx[:, jc * 128:(jc + 1) * 128],
                                        ident[0:H, 0:H])
                    nc.scalar.copy(uro[:, jc * 17:jc * 17 + H], ps[:])
                    nc.vector.memset(uro[:, jc * 17 + H:jc * 17 + 17], 1.0)
                for it, (off, w) in enumerate(ITILES):
                    rro = (rro0, rro1)[it]
                    ps = pss.tile([w, H], f32, tag="tr")
                    nc.tensor.transpose(ps[:], rts[:, off:off + w], ident[0:H, 0:H])
                    nc.scalar.copy(rro[:, 0:H], ps[:])
                    nc.vector.memset(rro[:, H:17], 1.0)
                    ps = pss.tile([w, H], f32, tag="tr")
                    nc.tensor.transpose(ps[:], uts[:, off:off + w], ident[0:H, 0:H])
                    nc.scalar.copy((urs0, urs1)[it][:], ps[:])

            with (
                tc.tile_pool(name="psA", bufs=4, space="PSUM") as psA,
                tc.tile_pool(name="psB", bufs=2, space="PSUM") as psB,
                tc.tile_pool(name="psC", bufs=1, space="PSUM") as psC,
                tc.tile_pool(name="psD", bufs=1, space="PSUM") as psD,
            ):
                bsb = []
                for it, (off, w) in enumerate(ITILES):
                    ct = (c0, c1)[it]
                    for k in range(NJ3):
                        j0 = k * 512
                        sp = psA.tile([w, 512], f32, tag="sm")
                        nc.tensor.matmul(sp[:], Slhs[:, off:off + w],
                                         UTx[:, j0:j0 + 512], start=True, stop=False)
                        nc.tensor.matmul(sp[:], SlhsX[:, off:off + w],
                                         UTxX[:, j0:j0 + 512], start=False, stop=True)
                        mp = psA.tile([w, 512], f32, tag="sm")
                        nc.tensor.matmul(mp[:], mvwms[:, off:off + w],
                                         mvwm[:, j0:j0 + 512], start=True, stop=True)
                        sig = wp.tile([w, 512], f32, tag="sig")
                        nc.scalar.activation(sig[:], sp[:], AF.Sigmoid)
                        # softplus(S) = S + ln(1 + exp(-S)); exact identity,
                        # no overflow since S >= 0 (squared distance)
                        e1 = wp.tile([w, 512], f32, tag="e1")
                        nc.scalar.activation(e1[:], sp[:], AF.Exp, scale=-1.0)
                        l1 = wp.tile([w, 512], f32, tag="l1")
                        nc.scalar.activation(l1[:], e1[:], AF.Ln, bias=1.0)
                        dist = wp.tile([w, 512], f32, tag="dist")
                        nc.vector.tensor_add(dist[:], l1[:], sp[:])
                        inv = wp.tile([w, 512], f32, tag="inv")
                        nc.vector.reciprocal(inv[:], dist[:])
                        aux = wp.tile([w, 512], f32, tag="aux")
                        nc.scalar.activation(aux[:], inv[:], AF.Copy,
                                             bias=2.0, scale=-4.0)
                        i2 = wp.tile([w, 512], f32, tag="i2")
                        nc.vector.tensor_mul(i2[:], inv[:], inv[:])
                        w2 = wp.tile([w, 512], f32, tag="w2")
                        nc.vector.tensor_mul(w2[:], i2[:], aux[:])
                        sm_ = wp.tile([w, 512], f32, tag="smt")
                        nc.vector.tensor_mul(sm_[:], sig[:], mp[:])
                        nc.vector.tensor_mul(ct[:, j0:j0 + 512], w2[:], sm_[:])
                    # B_part = C_shard @ [U | 1]  (transpose C chunks on PE)
                    bp = psC.tile([w, 17], f32, tag="acc")
                    for jc in range(NJ):
                        tp = psB.tile([128, w], f32, tag="ct")
                        nc.tensor.transpose(tp[:], ct[:, jc * 128:(jc + 1) * 128],
                                            ident[0:w, 0:w])
                        tsb = wp.tile([128, w], f32, tag="tsb")
                        nc.scalar.copy(tsb[:], tp[:])
                        nc.tensor.matmul(bp[:], tsb[:], uro[:, jc * 17:(jc + 1) * 17],
                                         start=(jc == 0), stop=(jc == NJ - 1))
                    bs = wp.tile([w, 17], f32, tag="bsb")
                    nc.vector.tensor_copy(bs[:], bp[:])
                    bsb.append(bs)

                # P_part[j] = sum_{i in shard} c_ij * [r_i | 1]
                for jc in range(NJ):
                    pp = psD.tile([128, 17], f32, tag="p")
                    nc.tensor.matmul(pp[:], c0[:, jc * 128:(jc + 1) * 128], rro0[:],
                                     start=True, stop=False)
                    nc.tensor.matmul(pp[:], c1[:, jc * 128:(jc + 1) * 128], rro1[:],
                                     start=False, stop=True)
                    psb_ = wp.tile([128, 17], f32, tag="psb")
                    nc.vector.tensor_copy(psb_[:], pp[:])
                    nc.sync.dma_start(P_dram[jc * 128:(jc + 1) * 128, :], psb_[:])

                nc.gpsimd.collective_compute(
                    "ReduceScatter",
                    mybir.AluOpType.add,
                    replica_groups=[list(range(NCORES))],
                    ins=[P_dram.opt()],
                    outs=[P_red.opt()],
                )

                for it, (off, w) in enumerate(ITILES):
                    urs, rro, bs = (urs0, urs1)[it], (rro0, rro1)[it], bsb[it]
                    pr = wp.tile([w, 17], f32, tag="pr")
                    nc.sync.dma_start(pr[:], P_red[off:off + w, :])
                    # A = ccol*u - CtR ; B = CU - crow*r ; D = A - B
                    a_t = wp.tile([w, H], f32, tag="a_t")
                    nc.vector.tensor_scalar_mul(a_t[:], urs[:], pr[:, H:17])
                    nc.vector.tensor_sub(a_t[:], a_t[:], pr[:, 0:H])
                    b_t = wp.tile([w, H], f32, tag="b_t")
                    nc.vector.tensor_scalar_mul(b_t[:], rro[:, 0:H], bs[:, H:17])
                    d_t = wp.tile([w, H], f32, tag="d_t")
                    nc.vector.tensor_sub(d_t[:], bs[:, 0:H], b_t[:])
                    nc.vector.tensor_sub(d_t[:], a_t[:], d_t[:])
                    dtp = psB.tile([H, w], f32, tag="ct")
                    nc.tensor.transpose(dtp[:], d_t[:], ident[0:w, 0:w])
                    dts = wp.tile([H, w], f32, tag="dts")
                    nc.scalar.copy(dts[:], dtp[:])
                    hq = psC.tile([w, 32], f32, tag="acc")
                    nc.tensor.matmul(hq[:], dts[:], W1q[:], start=True, stop=True)

                    # kinetic -> dq
                    m_t = wp.tile([w, 1], f32, tag="m_t")
                    nc.sync.dma_start(m_t[:], m_d[off:off + w, :])
                    mi2 = wp.tile([w, 1], f32, tag="mi2")
                    nc.vector.reciprocal(mi2[:], m_t[:])
                    nc.scalar.mul(mi2[:], mi2[:], 2.0)
                    zt = psB.tile([w, H], f32, tag="ct")
                    nc.tensor.matmul(zt[:], vpTs[:, off:off + w], WTT[:],
                                     start=True, stop=True)
                    et = wp.tile([w, H], f32, tag="et")
                    nc.scalar.activation(et[:], zt[:], AF.Exp, scale=-1.0)
                    lt = wp.tile([w, H], f32, tag="lt")
                    nc.scalar.activation(lt[:], et[:], AF.Ln, bias=1.0)
                    pw = wp.tile([w, H], f32, tag="pw")
                    nc.vector.tensor_add(pw[:], lt[:], zt[:])
                    sg = wp.tile([w, H], f32, tag="sg")
                    nc.scalar.activation(sg[:], zt[:], AF.Sigmoid)
                    gz = wp.tile([w, H], f32, tag="gz")
                    nc.vector.tensor_mul(gz[:], pw[:], sg[:])
                    nc.vector.tensor_scalar_mul(gz[:], gz[:], mi2[:])
                    gtp = psB.tile([H, w], f32, tag="ct")
                    nc.tensor.transpose(gtp[:], gz[:], ident[0:w, 0:w])
                    gts = wp.tile([H, w], f32, tag="gts")
                    nc.scalar.copy(gts[:], gtp[:])
                    dqp = psD.tile([w, 32], f32, tag="p")
                    nc.tensor.matmul(dqp[:], gts[:], WTp[:], start=True, stop=True)
                    dqs = wp.tile([w, 32], f32, tag="dqs")
                    nc.vector.tensor_copy(dqs[:], dqp[:])
                    nc.sync.dma_start(dq_d[off:off + w, :], dqs[:])

                    # dissipated -> dp
                    zf = psB.tile([w, H], f32, tag="ct")
                    nc.tensor.matmul(zf[:], pTs[:, off:off + w], WFT[:],
                                     start=True, stop=True)
                    ef = wp.tile([w, H], f32, tag="ef")
                    nc.scalar.activation(ef[:], zf[:], AF.Exp, scale=-1.0)
                    lf = wp.tile([w, H], f32, tag="lf")
                    nc.scalar.activation(lf[:], ef[:], AF.Ln, bias=1.0)
                    pwf = wp.tile([w, H], f32, tag="pwf")
                    nc.vector.tensor_add(pwf[:], lf[:], zf[:])
                    sgf = wp.tile([w, H], f32, tag="sgf")
                    nc.scalar.activation(sgf[:], zf[:], AF.Sigmoid)
                    gf = wp.tile([w, H], f32, tag="gf")
                    nc.vector.tensor_mul(gf[:], pwf[:], sgf[:])
                    nc.vector.tensor_scalar_mul(gf[:], gf[:], mi2[:])
                    gfp = psB.tile([H, w], f32, tag="ct")
                    nc.tensor.transpose(gfp[:], gf[:], ident[0:w, 0:w])
                    gfs = wp.tile([H, w], f32, tag="gfs")
                    nc.scalar.copy(gfs[:], gfp[:])
                    ddp = psD.tile([w, 32], f32, tag="p")
                    nc.tensor.matmul(ddp[:], gfs[:], WFm[:], start=True, stop=True)
                    hqs = wp.tile([w, 32], f32, tag="hqs")
                    nc.scalar.copy(hqs[:], hq[:])
                    dpsum = wp.tile([w, 32], f32, tag="dpsum")
                    nc.vector.tensor_add(dpsum[:], hqs[:], ddp[:])
                    dpo = wp.tile([w, 32], f32, tag="dpo")
                    nc.scalar.mul(dpo[:], dpsum[:], -1.0)
                    nc.sync.dma_start(dp_d[off:off + w, :], dpo[:])

    nc.finalize()
    return nc


def kernel(v, e, m, p, q, mvw, W_T, W1_w, W1_b, W_F):
    from concourse.bass_utils import run_bass_kernel_spmd

    f32 = np.float32
    v, m, p, q, mvw = (np.asarray(x, f32) for x in (v, m, p, q, mvw))
    W_T, W1_w, W1_b, W_F = (np.asarray(x, f32) for x in (W_T, W1_w, W1_b, W_F))

    vs = (1.0 / (1.0 + np.exp(-v))).astype(f32)
    vqT = np.ascontiguousarray(np.concatenate([vs, q], axis=1).T)    # [96,N]
    vpT = np.ascontiguousarray(np.concatenate([vs, p], axis=1).T)    # [96,N]
    pT = np.ascontiguousarray(p.T)                                   # [32,N]
    mvwm = np.ascontiguousarray(mvw * m[:, 0][None, :])              # [48,N]

    shared = {
        "vqT": vqT,
        "mvwm": mvwm,
        "W1wT": np.ascontiguousarray(W1_w.T),
        "W1b": np.ascontiguousarray(W1_b.reshape(H, 1)),
        "W1q": np.ascontiguousarray(W1_w[:, VD:]),
        "WTT": np.ascontiguousarray(W_T.T),
        "WTp": np.ascontiguousarray(W_T[:, VD:]),
        "WFT": np.ascontiguousarray(W_F.T),
        "WFm": np.ascontiguousarray(W_F),
        "ident": np.eye(128, dtype=f32),
        "ones_row": np.ones((1, N), dtype=f32),
    }
    in_maps = []
    for c in range(NCORES):
        sl = slice(c * SH, (c + 1) * SH)
        in_maps.append({
            **shared,
            "vqTs": np.ascontiguousarray(vqT[:, sl]),
            "vpTs": np.ascontiguousarray(vpT[:, sl]),
            "pTs": np.ascontiguousarray(pT[:, sl]),
            "m_s": np.ascontiguousarray(m[sl]),
            "mvwms": np.ascontiguousarray(mvwm[:, sl]),
        })

    if "nc" not in _CACHE:
        _CACHE["nc"] = _build_nc()
    nc = _CACHE["nc"]

    trace = bool(os.environ.get("BASS_KERNEL_TRACE"))
    if trace:
        try:
            from antenv.axon_hooks import get_axon_ntff_profile_hook  # noqa: F401
        except ImportError:
            trace = False
    res = run_bass_kernel_spmd(nc, in_maps, list(range(NCORES)), trace=trace)
    if trace and res.exec_time_ns is not None:
        print(f"HW exec time: {res.exec_time_ns} ns")

    dp = np.concatenate([res.results[c]["dp_s"] for c in range(NCORES)], axis=0)
    dq = np.concatenate([res.results[c]["dq_s"] for c in range(NCORES)], axis=0)
    return dp, dq


# revision 19
# speedup vs baseline: 455.9191x; 455.9191x over previous
"""Dissipative Hamiltonian derivation — Trainium2 Bass kernel, 8-core SPMD.

Math (derived analytically from the jax reference; gradients computed in
closed form, no autodiff):
  vs = sigmoid(v); vq = [vs, q]; R = vq @ W1_w.T; U = R + b
  S[i,j] = ||r_i||^2 + ||u_j||^2 - 2 r_i.u_j          (= ||u_j - r_i||^2)
  dist = softplus(S); C = 2*mask*(dist^-2 - 2*dist^-3)*sigmoid(S)
  mask = (mvw*m).T @ (mvw*m)
  B[i] = (C @ U)[i] - rowsum(C)[i]*r_i        (local to the row shard)
  A[j] = colsum(C)[j]*u_j - (C.T @ R)[j]      (needs cross-core reduction)
  dHdq = (A - B) @ W1_w[:, 64:]
  (diagonal of C cancels exactly in A - B, so it is never zeroed)
  dq = dHdp = (2/m)*(softplus(zT)*sigmoid(zT)) @ W_T[:, 64:],  zT = [vs,p]@W_T.T
  dp = -(dHdq + (2/m)*(softplus(zF)*sigmoid(zF)) @ W_F),        zF = p@W_F.T

Sharding: rows of the N^2 pairwise computation, 192 rows per core.
Each core computes C for its 192 rows, reduces B locally, and contributes
P[j] = sum_{i in shard} c_ij*[r_i | 1] which is ReduceScatter-summed so
core c receives the slab of A-partials for its own 192 output rows.
"""

import os
import numpy as np

N = 1536
NCORES = 8
SH = N // NCORES            # 192 rows per core
H = 16
VD = 64
ITILES = [(0, 128), (128, 64)]   # i-tiles inside a shard (partition dim <= 128)
NJ = N // 128                # 12 j-chunks of 128
NJ3 = N // 512               # 3 j-chunks of 512

_CACHE = {}


def _build_nc():
    from concourse import bacc, mybir
    import concourse.tile as tile

    f32 = mybir.dt.float32
    AF = mybir.ActivationFunctionType

    # Bacc (not raw Bass): its compile() pipeline splits multi-sem waits
    # (move_matmul_waits_to_ldweights / generate_event_semaphores), which
    # TRN2 codegen requires for Tile-generated programs.
    nc = bacc.Bacc(None, num_devices=NCORES)

    def ein(name, shape):
        return nc.dram_tensor(name, shape, f32, kind="ExternalInput")

    vqT_d = ein("vqT", [96, N])       # [vs; q].T replicated
    vqTs_d = ein("vqTs", [96, SH])    # shard columns of vqT
    vpTs_d = ein("vpTs", [96, SH])    # [vs; p].T shard columns
    pTs_d = ein("pTs", [32, SH])
    m_d = ein("m_s", [SH, 1])
    mvwm_d = ein("mvwm", [48, N])     # mvw * m (mask factor), replicated
    mvwms_d = ein("mvwms", [48, SH])
    W1wT_d = ein("W1wT", [96, H])
    W1b_d = ein("W1b", [H, 1])
    W1q_d = ein("W1q", [H, 32])
    WTT_d = ein("WTT", [96, H])
    WTp_d = ein("WTp", [H, 32])
    WFT_d = ein("WFT", [32, H])
    WFm_d = ein("WFm", [H, 32])
    id_d = ein("ident", [128, 128])
    ones_d = ein("ones_row", [1, N])

    dp_d = nc.dram_tensor("dp_s", [SH, 32], f32, kind="ExternalOutput")
    dq_d = nc.dram_tensor("dq_s", [SH, 32], f32, kind="ExternalOutput")

    with tile.TileContext(nc) as tc:
        with (
            tc.tile_pool(name="const", bufs=1) as cp,
            tc.tile_pool(name="work", bufs=3) as wp,
            tc.tile_pool(name="dram", bufs=1, space="DRAM") as drp,
        ):
            def load(d, shape, tag):
                t = cp.tile(shape, f32, tag=tag)
                nc.sync.dma_start(t[:], d[:])
                return t

            vqT = load(vqT_d, [96, N], "vqT")
            vqTs = load(vqTs_d, [96, SH], "vqTs")
            vpTs = load(vpTs_d, [96, SH], "vpTs")
            pTs = load(pTs_d, [32, SH], "pTs")
            mvwm = load(mvwm_d, [48, N], "mvwm")
            mvwms = load(mvwms_d, [48, SH], "mvwms")
            W1wT = load(W1wT_d, [96, H], "W1wT")
            W1b = load(W1b_d, [H, 1], "W1b")
            W1q = load(W1q_d, [H, 32], "W1q")
            WTT = load(WTT_d, [96, H], "WTT")
            WTp = load(WTp_d, [H, 32], "WTp")
            WFT = load(WFT_d, [32, H], "WFT")
            WFm = load(WFm_d, [H, 32], "WFm")
            ident = load(id_d, [128, 128], "ident")

            UTx = cp.tile([H, N], f32, tag="UTx")      # U.T
            UTxX = cp.tile([2, N], f32, tag="UTxX")    # [ones; un2]
            ut2 = cp.tile([H, N], f32, tag="ut2")
            Slhs = cp.tile([H, SH], f32, tag="Slhs")   # -2 R.T
            SlhsX = cp.tile([2, SH], f32, tag="SlhsX")  # [rn2; ones]
            ones16 = cp.tile([H, 1], f32, tag="ones16")
            rts = cp.tile([H, SH], f32, tag="rts")     # R.T shard cols
            uts = cp.tile([H, SH], f32, tag="uts")     # U.T shard cols
            uro = cp.tile([128, 17 * NJ], f32, tag="uro")  # U rows | 1, per j-chunk
            rro0 = cp.tile([128, 17], f32, tag="rro0")     # R rows | 1, shard
            rro1 = cp.tile([64, 17], f32, tag="rro1")
            urs0 = cp.tile([128, H], f32, tag="urs0")      # U rows, shard
            urs1 = cp.tile([64, H], f32, tag="urs1")
            c0 = cp.tile([128, N], f32, tag="c0")
            c1 = cp.tile([64, N], f32, tag="c1")

            P_dram = drp.tile([N, 17], f32)
            P_red = drp.tile([SH, 17], f32)

            nc.vector.memset(ones16[:], 1.0)

            with tc.tile_pool(name="pss", bufs=4, space="PSUM") as pss:
                # U.T = R.T + b, full N
                for k in range(NJ3):
                    ps = pss.tile([H, 512], f32, tag="set")
                    nc.tensor.matmul(ps[:], W1wT[:], vqT[:, k * 512:(k + 1) * 512],
                                     start=True, stop=True)
                    nc.vector.tensor_scalar_add(UTx[:, k * 512:(k + 1) * 512],
                                                ps[:], W1b[:])
                # auxiliary rows [ones; un2] live in their own 2-partition tile
                # (DMA-written: partition 1 is off the quad boundary)
                nc.sync.dma_start(UTxX[0:1, :], ones_d[:, :])
                # un2 row = colwise ||u||^2
                nc.vector.tensor_mul(ut2[:], UTx[:], UTx[:])
                for k in range(NJ3):
                    ps = pss.tile([1, 512], f32, tag="set")
                    nc.tensor.matmul(ps[:], ones16[:], ut2[:, k * 512:(k + 1) * 512],
                                     start=True, stop=True)
                    tmp = wp.tile([1, 512], f32, tag="row")
                    nc.scalar.copy(tmp[:], ps[:])
                    nc.sync.dma_start(UTxX[1:2, k * 512:(k + 1) * 512], tmp[:])
                # R.T shard cols
                ps = pss.tile([H, SH], f32, tag="set")
                nc.tensor.matmul(ps[:], W1wT[:], vqTs[:], start=True, stop=True)
                nc.scalar.copy(rts[:], ps[:])
                nc.vector.tensor_scalar_add(uts[:], rts[:], W1b[:])
                # S lhsT main = -2 R.T ; aux rows = [rn2; ones]
                nc.scalar.mul(Slhs[:], rts[:], -2.0)
                rts2 = wp.tile([H, SH], f32, tag="rts2")
                nc.vector.tensor_mul(rts2[:], rts[:], rts[:])
                ps = pss.tile([1, SH], f32, tag="set")
                nc.tensor.matmul(ps[:], ones16[:], rts2[:], start=True, stop=True)
                tmp = wp.tile([1, SH], f32, tag="row2")
                nc.scalar.copy(tmp[:], ps[:])
                nc.sync.dma_start(SlhsX[0:1, :], tmp[:])
                nc.sync.dma_start(SlhsX[1:2, :], ones_d[:, 0:SH])
                # U rows (all N, by 128-chunk) and R/U rows for the shard
                for jc in range(NJ):
                    ps = pss.tile([128, H], f32, tag="tr")
                    nc.tensor.transpose(ps[:], UTx[:, jc * 128:(jc + 1) * 128],
                                        ident[0:H, 0:H])
                    nc.scalar.copy(uro[:, jc * 17:jc * 17 + H], ps[:])
                    nc.vector.memset(uro[:, jc * 17 + H:jc * 17 + 17], 1.0)
                for it, (off, w) in enumerate(ITILES):
                    rro = (rro0, rro1)[it]
                    ps = pss.tile([w, H], f32, tag="tr")
                    nc.tensor.transpose(ps[:], rts[:, off:off + w], ident[0:H, 0:H])
                    nc.scalar.copy(rro[:, 0:H], ps[:])
                    nc.vector.memset(rro[:, H:17], 1.0)
                    ps = pss.tile([w, H], f32, tag="tr")
                    nc.tensor.transpose(ps[:], uts[:, off:off + w], ident[0:H, 0:H])
                    nc.scalar.copy((urs0, urs1)[it][:], ps[:])

            with (
                tc.tile_pool(name="psA", bufs=4, space="PSUM") as psA,
                tc.tile_pool(name="psB", bufs=2, space="PSUM") as psB,
                tc.tile_pool(name="psC", bufs=1, space="PSUM") as psC,
                tc.tile_pool(name="psD", bufs=1, space="PSUM") as psD,
            ):
                bsb = []
                for it, (off, w) in enumerate(ITILES):
                    ct = (c0, c1)[it]
                    for k in range(NJ3):
                        j0 = k * 512
                        sp = psA.tile([w, 512], f32, tag="sm")
                        nc.tensor.matmul(sp[:], Slhs[:, off:off + w],
                                         UTx[:, j0:j0 + 512], start=True, stop=False)
                        nc.tensor.matmul(sp[:], SlhsX[:, off:off + w],
                                         UTxX[:, j0:j0 + 512], start=False, stop=True)
                        mp = psA.tile([w, 512], f32, tag="sm")
                        nc.tensor.matmul(mp[:], mvwms[:, off:off + w],
                                         mvwm[:, j0:j0 + 512], start=True, stop=True)
                        sig = wp.tile([w, 512], f32, tag="sig")
                        nc.scalar.activation(sig[:], sp[:], AF.Sigmoid)
                        # softplus(S) = S + ln(1 + exp(-S)); exact identity,
                        # no overflow since S >= 0 (squared distance)
                        e1 = wp.tile([w, 512], f32, tag="e1")
                        nc.scalar.activation(e1[:], sp[:], AF.Exp, scale=-1.0)
                        l1 = wp.tile([w, 512], f32, tag="l1")
                        nc.scalar.activation(l1[:], e1[:], AF.Ln, bias=1.0)
                        dist = wp.tile([w, 512], f32, tag="dist")
                        nc.vector.tensor_add(dist[:], l1[:], sp[:])
                        inv = wp.tile([w, 512], f32, tag="inv")
                        nc.vector.reciprocal(inv[:], dist[:])
                        aux = wp.tile([w, 512], f32, tag="aux")
                        nc.scalar.activation(aux[:], inv[:], AF.Copy,
                                             bias=2.0, scale=-4.0)
                        i2 = wp.tile([w, 512], f32, tag="i2")
                        nc.vector.tensor_mul(i2[:], inv[:], inv[:])
                        w2 = wp.tile([w, 512], f32, tag="w2")
                        nc.vector.tensor_mul(w2[:], i2[:], aux[:])
                        sm_ = wp.tile([w, 512], f32, tag="smt")
                        nc.vector.tensor_mul(sm_[:], sig[:], mp[:])
                        nc.vector.tensor_mul(ct[:, j0:j0 + 512], w2[:], sm_[:])
                    # B_part = C_shard @ [U | 1]  (transpose C chunks on PE)
                    bp = psC.tile([w, 17], f32, tag="acc")
                    for jc in range(NJ):
                        tp = psB.tile([128, w], f32, tag="ct")
                        nc.tensor.transpose(tp[:], ct[:, jc * 128:(jc + 1) * 128],
                                            ident[0:w, 0:w])
                        tsb = wp.tile([128, w], f32, tag="tsb")
                        nc.scalar.copy(tsb[:], tp[:])
                        nc.tensor.matmul(bp[:], tsb[:], uro[:, jc * 17:(jc + 1) * 17],
                                         start=(jc == 0), stop=(jc == NJ - 1))
                    bs = wp.tile([w, 17], f32, tag="bsb")
                    nc.vector.tensor_copy(bs[:], bp[:])
                    bsb.append(bs)

                # P_part[j] = sum_{i in shard} c_ij * [r_i | 1]
                for jc in range(NJ):
                    pp = psD.tile([128, 17], f32, tag="p")
                    nc.tensor.matmul(pp[:], c0[:, jc * 128:(jc + 1) * 128], rro0[:],
                                     start=True, stop=False)
                    nc.tensor.matmul(pp[:], c1[:, jc * 128:(jc + 1) * 128], rro1[:],
                                     start=False, stop=True)
                    psb_ = wp.tile([128, 17], f32, tag="psb")
                    nc.vector.tensor_copy(psb_[:], pp[:])
                    nc.sync.dma_start(P_dram[jc * 128:(jc + 1) * 128, :], psb_[:])

                nc.gpsimd.collective_compute(
                    "ReduceScatter",
                    mybir.AluOpType.add,
                    replica_groups=[list(range(NCORES))],
                    ins=[P_dram.opt()],
                    outs=[P_red.opt()],
                )

                for it, (off, w) in enumerate(ITILES):
                    urs, rro, bs = (urs0, urs1)[it], (rro0, rro1)[it], bsb[it]
                    pr = wp.tile([w, 17], f32, tag="pr")
                    nc.sync.dma_start(pr[:], P_red[off:off + w, :])
                    # A = ccol*u - CtR ; B = CU - crow*r ; D = A - B
                    a_t = wp.tile([w, H], f32, tag="a_t")
                    nc.vector.tensor_scalar_mul(a_t[:], urs[:], pr[:, H:17])
                    nc.vector.tensor_sub(a_t[:], a_t[:], pr[:, 0:H])
                    b_t = wp.tile([w, H], f32, tag="b_t")
                    nc.vector.tensor_scalar_mul(b_t[:], rro[:, 0:H], bs[:, H:17])
                    d_t = wp.tile([w, H], f32, tag="d_t")
                    nc.vector.tensor_sub(d_t[:], bs[:, 0:H], b_t[:])
                    nc.vector.tensor_sub(d_t[:], a_t[:], d_t[:])
                    dtp = psB.tile([H, w], f32, tag="ct")
                    nc.tensor.transpose(dtp[:], d_t[:], ident[0:w, 0:w])
                    dts = wp.tile([H, w], f32, tag="dts")
                    nc.scalar.copy(dts[:], dtp[:])
                    hq = psC.tile([w, 32], f32, tag="acc")
                    nc.tensor.matmul(hq[:], dts[:], W1q[:], start=True, stop=True)

                    # kinetic -> dq
                    m_t = wp.tile([w, 1], f32, tag="m_t")
                    nc.sync.dma_start(m_t[:], m_d[off:off + w, :])
                    mi2 = wp.tile([w, 1], f32, tag="mi2")
                    nc.vector.reciprocal(mi2[:], m_t[:])
                    nc.scalar.mul(mi2[:], mi2[:], 2.0)
                    zt = psB.tile([w, H], f32, tag="ct")
                    nc.tensor.matmul(zt[:], vpTs[:, off:off + w], WTT[:],
                                     start=True, stop=True)
                    et = wp.tile([w, H], f32, tag="et")
                    nc.scalar.activation(et[:], zt[:], AF.Exp, scale=-1.0)
                    lt = wp.tile([w, H], f32, tag="lt")
                    nc.scalar.activation(lt[:], et[:], AF.Ln, bias=1.0)
                    pw = wp.tile([w, H], f32, tag="pw")
                    nc.vector.tensor_add(pw[:], lt[:], zt[:])
                    sg = wp.tile([w, H], f32, tag="sg")
                    nc.scalar.activation(sg[:], zt[:], AF.Sigmoid)
                    gz = wp.tile([w, H], f32, tag="gz")
                    nc.vector.tensor_mul(gz[:], pw[:], sg[:])
                    nc.vector.tensor_scalar_mul(gz[:], gz[:], mi2[:])
                    gtp = psB.tile([H, w], f32, tag="ct")
                    nc.tensor.transpose(gtp[:], gz[:], ident[0:w, 0:w])
                    gts = wp.tile([H, w], f32, tag="gts")
                    nc.scalar.copy(gts[:], gtp[:])
                    dqp = psD.tile([w, 32], f32, tag="p")
                    nc.tensor.matmul(dqp[:], gts[:], WTp[:], start=True, stop=True)
                    dqs = wp.tile([w, 32], f32, tag="dqs")
                    nc.vector.tensor_copy(dqs[:], dqp[:])
                    nc.sync.dma_start(dq_d[off:off + w, :], dqs[:])

                    # dissipated -> dp
                    zf = psB.tile([w, H], f32, tag="ct")
                    nc.tensor.matmul(zf[:], pTs[:, off:off + w], WFT[:],
                                     start=True, stop=True)
                    ef = wp.tile([w, H], f32, tag="ef")
                    nc.scalar.activation(ef[:], zf[:], AF.Exp, scale=-1.0)
                    lf = wp.tile([w, H], f32, tag="lf")
                    nc.scalar.activation(lf[:], ef[:], AF.Ln, bias=1.0)
                    pwf = wp.tile([w, H], f32, tag="pwf")
                    nc.vector.tensor_add(pwf[:], lf[:], zf[:])
                    sgf = wp.tile([w, H], f32, tag="sgf")
                    nc.scalar.activation(sgf[:], zf[:], AF.Sigmoid)
                    gf = wp.tile([w, H], f32, tag="gf")
                    nc.vector.tensor_mul(gf[:], pwf[:], sgf[:])
                    nc.vector.tensor_scalar_mul(gf[:], gf[:], mi2[:])
                    gfp = psB.tile([H, w], f32, tag="ct")
                    nc.tensor.transpose(gfp[:], gf[:], ident[0:w, 0:w])
                    gfs = wp.tile([H, w], f32, tag="gfs")
                    nc.scalar.copy(gfs[:], gfp[:])
                    ddp = psD.tile([w, 32], f32, tag="p")
                    nc.tensor.matmul(ddp[:], gfs[:], WFm[:], start=True, stop=True)
                    hqs = wp.tile([w, 32], f32, tag="hqs")
                    nc.scalar.copy(hqs[:], hq[:])
                    dpsum = wp.tile([w, 32], f32, tag="dpsum")
                    nc.vector.tensor_add(dpsum[:], hqs[:], ddp[:])
                    dpo = wp.tile([w, 32], f32, tag="dpo")
                    nc.scalar.mul(dpo[:], dpsum[:], -1.0)
                    nc.sync.dma_start(dp_d[off:off + w, :], dpo[:])

    nc.finalize()
    return nc


def _prepare_in_maps(v, e, m, p, q, mvw, W_T, W1_w, W1_b, W_F):
    f32 = np.float32
    v, m, p, q, mvw = (np.asarray(x, f32) for x in (v, m, p, q, mvw))
    W_T, W1_w, W1_b, W_F = (np.asarray(x, f32) for x in (W_T, W1_w, W1_b, W_F))

    vs = (1.0 / (1.0 + np.exp(-v))).astype(f32)
    vqT = np.ascontiguousarray(np.concatenate([vs, q], axis=1).T)    # [96,N]
    vpT = np.ascontiguousarray(np.concatenate([vs, p], axis=1).T)    # [96,N]
    pT = np.ascontiguousarray(p.T)                                   # [32,N]
    mvwm = np.ascontiguousarray(mvw * m[:, 0][None, :])              # [48,N]

    shared = {
        "vqT": vqT,
        "mvwm": mvwm,
        "W1wT": np.ascontiguousarray(W1_w.T),
        "W1b": np.ascontiguousarray(W1_b.reshape(H, 1)),
        "W1q": np.ascontiguousarray(W1_w[:, VD:]),
        "WTT": np.ascontiguousarray(W_T.T),
        "WTp": np.ascontiguousarray(W_T[:, VD:]),
        "WFT": np.ascontiguousarray(W_F.T),
        "WFm": np.ascontiguousarray(W_F),
        "ident": np.eye(128, dtype=f32),
        "ones_row": np.ones((1, N), dtype=f32),
    }
    in_maps = []
    for c in range(NCORES):
        sl = slice(c * SH, (c + 1) * SH)
        in_maps.append({
            **shared,
            "vqTs": np.ascontiguousarray(vqT[:, sl]),
            "vpTs": np.ascontiguousarray(vpT[:, sl]),
            "pTs": np.ascontiguousarray(pT[:, sl]),
            "m_s": np.ascontiguousarray(m[sl]),
            "mvwms": np.ascontiguousarray(mvwm[:, sl]),
        })
    return in_maps


def kernel(v, e, m, p, q, mvw, W_T, W1_w, W1_b, W_F):
    from concourse.bass_utils import run_bass_kernel_spmd

    in_maps = _prepare_in_maps(v, e, m, p, q, mvw, W_T, W1_w, W1_b, W_F)

    if "nc" not in _CACHE:
        _CACHE["nc"] = _build_nc()
    nc = _CACHE["nc"]

    trace = bool(os.environ.get("BASS_KERNEL_TRACE"))
    if trace:
        try:
            from antenv.axon_hooks import get_axon_ntff_profile_hook  # noqa: F401
        except ImportError:
            trace = False
    res = run_bass_kernel_spmd(nc, in_maps, list(range(NCORES)), trace=trace)
    if trace and res.exec_time_ns is not None:
        print(f"HW exec time: {res.exec_time_ns} ns")

    dp = np.concatenate([res.results[c]["dp_s"] for c in range(NCORES)], axis=0)
    dq = np.concatenate([res.results[c]["dq_s"] for c in range(NCORES)], axis=0)
    return dp, dq
